# revision 12
# baseline (speedup 1.0000x reference)
"""LSTM decoder (nn_Decoder) on 8 Trainium2 NeuronCores.

Strategy:
  - Replicate the sequential LSTM recurrence on all 8 cores (it is serial in T;
    batch=32 gives too little parallelism to shard profitably), shard the output
    head over the vocab dim: each core computes logits[:, :, c*4000:(c+1)*4000].
    Unshard = host-side concat; no device collectives.
  - Recurrence matmuls: out = gates[B=32, 4H] with stationary h^T [K,32]
    replicated into the PE array's 4 column groups (tile_position=(0,32j)) so 4
    weight streams run concurrently -> ~4x util at M=32.
  - Embedding projection (e @ W_ih^T) folded into the recurrence as 4 extra
    K-chunks of the same accumulation (they do not depend on h_t, so they
    pipeline into the previous step's elementwise tail).
  - Gate layout on PSUM: partition = 32j+b, free = gate*256+u (hidden j*256+u)
    -> full 128-lane elementwise; h returned to h^T layout by 2 PE transposes.
  - All matmuls bf16 inputs / fp32 PSUM; c, gates elementwise in fp32.

Host does only data movement / layout prep plus the h0 projection
(z @ W_h^T: 8.4 MFLOP of 320 GFLOP total) and the embedding gather.
"""

import numpy as np
import ml_dtypes
from contextlib import ExitStack

import concourse.bass as bass  # noqa: F401
import concourse.tile as tile
import concourse.bacc as bacc
import concourse.mybir as mybir
from concourse import bass_utils

BF16 = ml_dtypes.bfloat16
N_CORES = 8
B, T = 32, 128
VOCAB, D_EMB, Z_DIM, HID = 32000, 512, 256, 1024
VSH = VOCAB // N_CORES    # 4000 vocab per core
NTOK = B * T              # 4096 tokens; token index = t*32 + b
KH = HID // 128           # 8 hidden K-chunks
KE = D_EMB // 128         # 4 embedding K-chunks
KC = KH + KE              # 12
GW = 4 * HID              # 4096 gate width
NT_HEAD = 8               # vocab tiles per core in the head
NV = VSH // NT_HEAD       # 500
MT_HEAD = NTOK // 128     # 32 token tiles in the head
EPT = 8                   # steps per embedding prefetch tile

_NC_CACHE = {}


def _perm():
    # psum gate order n = j*1024 + gate*256 + u  ->  torch W column gate*1024 + j*256 + u
    j = np.arange(4)[:, None, None]
    gate = np.arange(4)[None, :, None]
    u = np.arange(256)[None, None, :]
    return (gate * 1024 + j * 256 + u).reshape(-1)


def _build(repeat=1):
    if repeat in _NC_CACHE:
        return _NC_CACHE[repeat]
    nc = bacc.Bacc("TRN2", debug=False, num_devices=N_CORES)
    dt = mybir.dt
    eT_d = nc.dram_tensor("eT", [128, T * KE * 32], dt.bfloat16, kind="ExternalInput").ap()
    h0T_d = nc.dram_tensor("h0T", [128, KH * B], dt.bfloat16, kind="ExternalInput").ap()
    Ws_d = nc.dram_tensor("Ws", [128, KC * GW], dt.bfloat16, kind="ExternalInput").ap()
    biasg_d = nc.dram_tensor("bias_g", [1, GW + 32], dt.bfloat16, kind="ExternalInput").ap()
    id_d = nc.dram_tensor("ident", [128, 128], dt.bfloat16, kind="ExternalInput").ap()
    WoT_d = nc.dram_tensor("WoT", [128, NT_HEAD * KH * NV], dt.bfloat16, kind="ExternalInput").ap()
    biaso_d = nc.dram_tensor("bias_o", [128, VSH], dt.bfloat16, kind="ExternalInput").ap()
    out_d = nc.dram_tensor("out", [NTOK, VSH], dt.float32, kind="ExternalOutput").ap()

    ACT = mybir.ActivationFunctionType

    with tile.TileContext(nc) as tc, ExitStack() as ctx:
        pers = ctx.enter_context(tc.tile_pool(name="pers", bufs=1))
        hsT = pers.tile([128, KH * NTOK], dt.bfloat16)  # h^T for all steps; 64KB/part
        ident = pers.tile([128, 128], dt.bfloat16)
        nc.sync.dma_start(ident[:], id_d)
        # col = ((2j+m)*4096) + token
        hsT_v = hsT[:].rearrange("p (j m n) -> p j m n", j=4, m=2)

        for _rep in range(repeat):
            _emit_body(nc, tc, hsT, hsT_v, ident,
                       eT_d, h0T_d, Ws_d, biasg_d, WoT_d, biaso_d, out_d)
    nc.compile()
    _NC_CACHE[repeat] = nc
    return nc


def _emit_body(nc, tc, hsT, hsT_v, ident,
               eT_d, h0T_d, Ws_d, biasg_d, WoT_d, biaso_d, out_d):
        dt = mybir.dt
        ACT = mybir.ActivationFunctionType
        with ExitStack() as rctx:
            wpool = rctx.enter_context(tc.tile_pool(name="ws", bufs=1))
            ws = wpool.tile([128, KC * GW], dt.bfloat16)  # 96KB/part
            nc.sync.dma_start(ws[:], Ws_d)
            bias_g = wpool.tile([1, GW + 32], dt.bfloat16)
            nc.sync.dma_start(bias_g[:], biasg_d)
            h0T_s = wpool.tile([128, KH * B], dt.bfloat16)
            nc.sync.dma_start(h0T_s[:], h0T_d)
            epool = rctx.enter_context(tc.tile_pool(name="eT", bufs=3))
            gpsum = rctx.enter_context(tc.tile_pool(name="gps", bufs=2, space="PSUM"))
            tpsum = rctx.enter_context(tc.tile_pool(name="tps", bufs=2, space="PSUM"))
            ew = rctx.enter_context(tc.tile_pool(name="ew", bufs=2))
            cpool = rctx.enter_context(tc.tile_pool(name="cst", bufs=1))

            c_sb = cpool.tile([128, 256], dt.float32)
            nc.vector.memset(c_sb[:], 0.0)

            def mm_block(g, lhsT, k, start, stop):
                for nh in range(2):
                    for j in range(4):
                        nc.tensor.matmul(
                            g[32 * j:32 * j + 32, nh * 512:(nh + 1) * 512],
                            lhsT,
                            ws[:, k * GW + j * 1024 + nh * 512:
                               k * GW + j * 1024 + (nh + 1) * 512],
                            start=start, stop=stop,
                            tile_position=(0, 32 * j),
                        )

            def emit_e_part(g, t, et):
                # gate bias as a K=1 rank-1 matmul (ones stationary, bias row
                # streamed) -- keeps the bias add off the DVE critical chain.
                for nh in range(2):
                    for j in range(4):
                        nc.tensor.matmul(
                            g[32 * j:32 * j + 32, nh * 512:(nh + 1) * 512],
                            bias_g[:, GW:GW + 32],
                            bias_g[:, j * 1024 + nh * 512:j * 1024 + (nh + 1) * 512],
                            start=True, stop=False,
                            tile_position=(0, 32 * j),
                        )
                # e-chunks: independent of h_t; emitted one step ahead so the
                # PE fills the elementwise-tail gap of the previous step.
                for cix in range(KE):
                    off = (t % EPT) * 128 + cix * 32
                    mm_block(g, et[:, off:off + 32], KH + cix, False, False)

            et = None
            g_cur = None
            for t in range(T):
                if t % EPT == 0:
                    et = epool.tile([128, EPT * 128], dt.bfloat16, tag="et")
                    nc.sync.dma_start(et[:], eT_d[:, t * 128:(t + EPT) * 128])
                if t == 0:
                    g_cur = gpsum.tile([128, 1024], dt.float32, tag="g")
                    emit_e_part(g_cur, 0, et)
                g = g_cur
                for k in range(KH):
                    if t == 0:
                        lhsT = h0T_s[:, k * 32:(k + 1) * 32]
                    else:
                        off = k * NTOK + (t - 1) * 32
                        lhsT = hsT[:, off:off + 32]
                    mm_block(g, lhsT, k, False, k == KH - 1)
                if t + 1 < T:
                    g_cur = gpsum.tile([128, 1024], dt.float32, tag="g")
                    et_next = et
                    if (t + 1) % EPT == 0:
                        et_next = epool.tile([128, EPT * 128], dt.bfloat16, tag="et")
                        nc.sync.dma_start(
                            et_next[:], eT_d[:, (t + 1) * 128:(t + 1 + EPT) * 128])
                        et = et_next
                    emit_e_part(g_cur, t + 1, et_next)
                if_sb = ew.tile([128, 512], dt.float32, tag="if")
                nc.scalar.activation(if_sb[:], g[:, 0:512], ACT.Sigmoid)
                gg_sb = ew.tile([128, 256], dt.float32, tag="gg")
                nc.scalar.activation(gg_sb[:], g[:, 512:768], ACT.Tanh)
                o_sb = ew.tile([128, 256], dt.float32, tag="o")
                nc.scalar.activation(o_sb[:], g[:, 768:1024], ACT.Sigmoid)
                nc.vector.tensor_mul(c_sb[:], c_sb[:], if_sb[:, 256:512])
                t1 = ew.tile([128, 256], dt.float32, tag="t1")
                nc.vector.tensor_mul(t1[:], if_sb[:, 0:256], gg_sb[:])
                nc.vector.tensor_add(c_sb[:], c_sb[:], t1[:])
                tc_sb = ew.tile([128, 256], dt.float32, tag="tc")
                nc.scalar.activation(tc_sb[:], c_sb[:], ACT.Tanh)
                h_bf = ew.tile([128, 256], dt.bfloat16, tag="h")
                nc.vector.tensor_mul(h_bf[:], o_sb[:], tc_sb[:])
                for m in range(2):
                    tr = tpsum.tile([128, 128], dt.bfloat16, tag="tr")
                    nc.tensor.transpose(tr[:], h_bf[:, m * 128:(m + 1) * 128], ident[:])
                    nc.vector.tensor_copy(
                        hsT_v[:, :, m, t * 32:(t + 1) * 32],
                        tr[:].rearrange("p (j b) -> p j b", j=4),
                    )

        with ExitStack() as hctx:
            wo_pool = hctx.enter_context(tc.tile_pool(name="wo", bufs=2))
            bo_pool = hctx.enter_context(tc.tile_pool(name="bo", bufs=1))
            hpsum = hctx.enter_context(tc.tile_pool(name="hps", bufs=4, space="PSUM"))
            opool = hctx.enter_context(tc.tile_pool(name="osb", bufs=4))
            bias_o = bo_pool.tile([128, VSH], dt.bfloat16)
            nc.sync.dma_start(bias_o[:], biaso_d)
            for nt in range(NT_HEAD):
                wo = wo_pool.tile([128, KH * NV], dt.bfloat16, tag="wo")
                nc.sync.dma_start(wo[:], WoT_d[:, nt * KH * NV:(nt + 1) * KH * NV])
                for mt in range(MT_HEAD):
                    ps = hpsum.tile([128, NV], dt.float32, tag="hp")
                    for k in range(KH):
                        nc.tensor.matmul(
                            ps[:],
                            hsT[:, k * NTOK + mt * 128:k * NTOK + (mt + 1) * 128],
                            wo[:, k * NV:(k + 1) * NV],
                            start=(k == 0), stop=(k == KH - 1),
                        )
                    osb = opool.tile([128, NV], dt.float32, tag="osb")
                    nc.vector.tensor_add(osb[:], ps[:], bias_o[:, nt * NV:(nt + 1) * NV])
                    nc.sync.dma_start(
                        out_d[mt * 128:(mt + 1) * 128, nt * NV:(nt + 1) * NV], osb[:])


def prep_in_maps(z, x, W_h, b_h, emb, W_ih, W_hh, b_ih, b_hh, W_out, b_out):
    f32 = np.float32
    z = np.asarray(z, f32)
    W_h = np.asarray(W_h, f32)
    b_h = np.asarray(b_h, f32)
    emb = np.asarray(emb, f32)
    W_ih = np.asarray(W_ih, f32)
    W_hh = np.asarray(W_hh, f32)
    b_ih = np.asarray(b_ih, f32)
    b_hh = np.asarray(b_hh, f32)
    W_out = np.asarray(W_out, f32)
    b_out = np.asarray(b_out, f32)
    x = np.asarray(x)

    h0 = np.tanh(z @ W_h.T + b_h)                       # [B, H]
    e = emb[x]                                          # [B, T, D]
    # eT[p, t*128 + c*32 + b] = e[b, t, c*128+p]
    eT = e.transpose(2, 1, 0).reshape(KE, 128, T, B)
    eT = np.ascontiguousarray(eT.transpose(1, 2, 0, 3)).reshape(128, T * KE * 32)
    # h0T[p, c*32+b] = h0[b, c*128+p]
    h0T = np.ascontiguousarray(h0.T.reshape(KH, 128, B).transpose(1, 0, 2)).reshape(128, KH * B)
    perm = _perm()
    Wcat = np.concatenate([W_hh.T, W_ih.T], axis=0)     # [H+D, 4H]
    Wp = Wcat[:, perm]
    Ws = np.ascontiguousarray(Wp.reshape(KC, 128, GW).transpose(1, 0, 2)).reshape(128, KC * GW)
    bsum = (b_ih + b_hh)[perm]
    bias_g = np.concatenate([bsum, np.ones(32, np.float32)]).reshape(1, GW + 32)
    ident = np.eye(128, dtype=BF16)

    base = {
        "eT": eT.astype(BF16),
        "h0T": h0T.astype(BF16),
        "Ws": Ws.astype(BF16),
        "bias_g": bias_g.astype(BF16),
        "ident": ident,
    }
    in_maps = []
    for c in range(N_CORES):
        Wsh = W_out[c * VSH:(c + 1) * VSH]              # [4000, 1024]
        WoT = np.ascontiguousarray(
            Wsh.reshape(NT_HEAD, NV, KH, 128).transpose(3, 0, 2, 1)
        ).reshape(128, NT_HEAD * KH * NV)
        bsh = b_out[c * VSH:(c + 1) * VSH]
        bias_o = np.ascontiguousarray(np.broadcast_to(bsh, (128, VSH)))
        m = dict(base)
        m["WoT"] = WoT.astype(BF16)
        m["bias_o"] = bias_o.astype(BF16)
        in_maps.append(m)
    return in_maps


def assemble(results):
    outs = [np.asarray(r["out"]).reshape(T, B, VSH) for r in results]
    full = np.concatenate(outs, axis=2)                 # [T, B, VOCAB]
    return np.ascontiguousarray(full.transpose(1, 0, 2))


def kernel(**inputs):
    in_maps = prep_in_maps(**inputs)
    nc = _build()
    res = bass_utils.run_bass_kernel_spmd(nc, in_maps, core_ids=list(range(N_CORES)))
    return assemble(res.results)
